# revision 15
# baseline (speedup 1.0000x reference)
"""CapsuleLayer (dynamic routing, ROUTING_ITER=2) Bass/Tile kernel for TRN2.

Contract: kernel(x, weight) takes FULL inputs
  x:      [64, 2048, 1, 16] f32
  weight: [1, 2048, 32, 16, 16] f32
returns FULL output [64, 32, 16] f32.

Sharding: data-parallel over batch B=64 across 8 cores (8 per core),
weight replicated. Self-contained: hardcodes shapes, imports only
numpy/ml_dtypes/concourse.

v2 layout/engine plan (per core, b_loc=8):
  u layout A: SBUF [p = g2*64 + b*8 + di, f = gh*512 + d*32 + j]
    where i = g*8 + di, g = 2*gh + g2  (G=256 groups, GH=128 pairs).
  phase 1 (DMA-bound ~95us): stream W chunks; build block-diag x
    stationaries on-chip (mask multiply, DVE); u-matmuls + interleaved
    s1 delta-matmuls on PE; PSUM->SBUF u copies mostly on Act.
  routing pass (x2): per chunk of CH=8 gh: prod=u*v (TT), d-tree,
    exp on Act, Z-reduce, c=eT*r, y=c*u; chunks 0..12 on DVE,
    13..15 on Pool(gpsimd); s accumulated with delta matmuls on PE.
"""

from contextlib import ExitStack

import ml_dtypes
import numpy as np

import concourse.bacc as bacc
import concourse.bass as bass
import concourse.mybir as mybir
import concourse.tile as tile
from concourse.bass_utils import run_bass_kernel_spmd

F32 = mybir.dt.float32
BF16 = mybir.dt.bfloat16
AF = mybir.ActivationFunctionType
AX = mybir.AxisListType

EPS = 1e-8
J, D, E = 32, 16, 16
JD = J * D  # 512


def emit_capsule(tc, w2, xc, d1, ds, mk, out, n_in, b_loc=8):
    """Emit the per-core capsule program.

    DRAM tensors (APs):
      w2  [G, 8, E, JD] bf16  weight, host-permuted to [g, di, e, (d j)]
      xc  [128, G, 8]   bf16  x compact: [(di,e), g, b]
      d1  [128, 8] bf16       delta matrix * (1/32)  (s1 accumulation)
      ds  [128, 8] bf16       delta matrix * 1.0     (s2/s3 accumulation)
      mk  [128, 8] bf16       block-diag mask: mk[(di,e), di'] = (di==di')
      out [b_loc, JD] f32     squash(s3) output, (d,j) layout
    """
    nc = tc.nc
    assert b_loc == 8
    G = n_in // 8          # 256
    GH = G // 2            # 128
    GDMA = 16              # groups per W DMA chunk
    NWCH = G // GDMA       # 16 W chunks
    CH = 8                 # gh per routing chunk
    NCH = GH // CH         # 16 routing chunks
    N_POOL = 3             # routing chunks on gpsimd (the last ones)

    ctx = ExitStack()
    singles = ctx.enter_context(tc.tile_pool(name="singles", bufs=1))
    small = ctx.enter_context(tc.tile_pool(name="small", bufs=1))
    dramp = ctx.enter_context(tc.tile_pool(name="dram_scratch", bufs=1, space="DRAM"))

    u_sb = singles.tile([128, GH * JD], BF16)
    d1_sb = singles.tile([128, 8], BF16)
    ds_sb = singles.tile([128, 8], BF16)
    mk_sb = singles.tile([128, 8], BF16)
    x_sb = singles.tile([128, G, 8], BF16)
    nc.sync.dma_start(out=d1_sb, in_=d1)
    nc.sync.dma_start(out=ds_sb, in_=ds)
    nc.sync.dma_start(out=mk_sb, in_=mk)
    nc.sync.dma_start(out=x_sb, in_=xc)
    v_exp = singles.tile([128, JD], BF16)
    V = singles.tile([8, JD], F32)      # running sum of v vectors
    s_sb = singles.tile([8, JD], F32)
    vscr = dramp.tile([8, JD], BF16)    # DRAM bounce buffer for v broadcast
    eps8 = singles.tile([8, 1], F32)
    nc.vector.memset(eps8, EPS)

    # ---------- squash helpers (all on 8 partitions, tiny) ----------
    def squash_j(s_in, v_out):
        # v = squash(s, axis=j):  sq[b,d] = sum_j s^2;  v = s*sq/((1+sq)*sqrt(sq+eps))
        t2 = small.tile([8, JD], F32, tag="sqt2")
        nc.vector.tensor_mul(t2, s_in, s_in)
        sv = small.tile([8, 4, J], F32, tag="sqv")
        sq, a, t3, w = sv[:, 0, :D], sv[:, 1, :D], sv[:, 2, :D], sv[:, 3, :D]
        nc.vector.reduce_sum(out=sq, in_=t2.rearrange("p (d j) -> p d j", d=D), axis=AX.X)
        nc.scalar.activation(a, sq, AF.Sqrt, bias=eps8)
        nc.vector.tensor_mul(t3, sq, a)
        nc.vector.tensor_add(t3, t3, a)          # a*(1+sq)
        nc.vector.reciprocal(w, t3)
        nc.vector.tensor_mul(w, w, sq)           # sq/((1+sq)a)
        wb = w.unsqueeze(2).broadcast_to([8, D, J])
        nc.vector.tensor_mul(v_out.rearrange("p (d j) -> p d j", d=D),
                             s_in.rearrange("p (d j) -> p d j", d=D), wb)

    def squash_d(s_in, v_out):
        # v = squash(s, axis=d): sq[b,j] = sum_d s^2
        t2 = small.tile([8, JD], F32, tag="sqt2")
        nc.vector.tensor_mul(t2, s_in, s_in)
        sv = small.tile([8, 4, J], F32, tag="sqv")
        sq, a, t3, w = sv[:, 0, :], sv[:, 1, :], sv[:, 2, :], sv[:, 3, :]
        nc.vector.reduce_sum(out=sq, in_=t2.rearrange("p (d j) -> p j d", d=D), axis=AX.X)
        nc.scalar.activation(a, sq, AF.Sqrt, bias=eps8)
        nc.vector.tensor_mul(t3, sq, a)
        nc.vector.tensor_add(t3, t3, a)
        nc.vector.reciprocal(w, t3)
        nc.vector.tensor_mul(w, w, sq)
        wb = w.unsqueeze(1).broadcast_to([8, D, J])
        nc.vector.tensor_mul(v_out.rearrange("p (d j) -> p d j", d=D),
                             s_in.rearrange("p (d j) -> p d j", d=D), wb)

    def refresh_v_exp():
        vb = small.tile([8, JD], BF16, tag="vb")
        nc.vector.tensor_copy(out=vb, in_=V)
        nc.sync.dma_start(out=vscr, in_=vb)
        src = vscr.unsqueeze(1).broadcast_to([8, 8, JD])
        for g2 in range(2):
            nc.sync.dma_start(out=v_exp[g2 * 64:(g2 + 1) * 64, :], in_=src)

    # ---------- phase 1: W pass (u = W @ x), s1 accumulation ----------
    with tc.tile_pool(name="wp", bufs=3) as wp, \
         tc.tile_pool(name="xbp", bufs=2) as xbp, \
         tc.tile_pool(name="up", bufs=3, space="PSUM") as up, \
         tc.tile_pool(name="sp", bufs=1, space="PSUM") as sp:
        s1_ps = sp.tile([8, JD], F32)
        mkb = mk_sb.unsqueeze(1).unsqueeze(1).broadcast_to([128, GDMA, 8, 8])
        for ci in range(NWCH):
            wt = wp.tile([128, GDMA, JD], BF16, tag="wt")
            # src: dims (k=(di,e) merged, g, jd)
            wsrc = w2[ci * GDMA:(ci + 1) * GDMA].rearrange("g di e f -> (di e) g f")
            nc.sync.dma_start(out=wt, in_=wsrc)
            # on-chip block-diag x stationary for this chunk:
            # xt[(di,e), g_loc, b, di'] = x[(di,e), g, b] * (di == di')
            xt = xbp.tile([128, GDMA, 8, 8], BF16, tag="xt")
            xb = x_sb[:, ci * GDMA:(ci + 1) * GDMA, :].unsqueeze(3) \
                .broadcast_to([128, GDMA, 8, 8])
            nc.gpsimd.tensor_mul(xt, xb, mkb)
            xt2 = xt.rearrange("p g b q -> p g (b q)")
            for gq in range(GDMA // 4):
                pt = up.tile([128, 2 * JD], F32, tag="upt")
                for idx in range(4):
                    gl = gq * 4 + idx
                    nc.tensor.matmul(
                        pt[(gl % 2) * 64:(gl % 2) * 64 + 64,
                           (idx // 2) * JD:(idx // 2) * JD + JD],
                        xt2[:, gl, :], wt[:, gl, :], start=True, stop=True)
                gh0 = (ci * GDMA) // 2 + gq * 2
                if gq % 4 == 0:
                    nc.vector.tensor_copy(out=u_sb[:, gh0 * JD:(gh0 + 2) * JD], in_=pt)
                else:
                    nc.scalar.copy(out=u_sb[:, gh0 * JD:(gh0 + 2) * JD], in_=pt)
                for gh in (gh0, gh0 + 1):
                    nc.tensor.matmul(s1_ps, d1_sb, u_sb[:, gh * JD:(gh + 1) * JD],
                                     start=(gh == 0), stop=(gh == GH - 1))
        nc.vector.tensor_copy(out=s_sb, in_=s1_ps)

    squash_j(s_sb, V)      # V = v1
    refresh_v_exp()

    # ---------- routing pass (T = u.V, softmax, s = sum_i c*u) ----------
    ND = NCH - N_POOL  # DVE routing chunks; pool gets the rest

    def routing_pass(final):
        # rp1: engine-local scratch (consumed by the next op on the same
        # in-order engine) -> single buffer per tag suffices.
        # ry: y tiles, consumed cross-engine by PE -> double buffered.
        # rp: small cross-engine tiles (eT from Act, softmax pieces).
        with tc.tile_pool(name="rp1", bufs=1) as rp1, \
             tc.tile_pool(name="ry", bufs=2) as ry, \
             tc.tile_pool(name="rp", bufs=2) as rp, \
             tc.tile_pool(name="pp1", bufs=1) as pp1, \
             tc.tile_pool(name="spp", bufs=1, space="PSUM") as spp:
            s_ps = spp.tile([8, JD], F32)
            smm = [0]  # emission-order counter for s-accumulation matmuls

            def u_ch_of(k):
                fs = k * CH * JD
                return u_sb[:, fs:fs + CH * JD].rearrange("p (g f) -> p g f", g=CH)

            def s_mms(k, y):
                for q in range(CH):
                    nc.tensor.matmul(s_ps, ds_sb,
                                     y[:, q].rearrange("p d j -> p (d j)"),
                                     start=(smm[0] == 0), stop=(smm[0] == GH - 1))
                    smm[0] += 1

            def front(k, ve, t_pool, tag, emit_exp=True):
                # prod = u * v, then tree-reduce over d (16->8->4->2->1), bf16
                u_ch = u_ch_of(k)
                prod = t_pool.tile([128, CH, JD], BF16, tag=tag + "prod")
                vb = v_exp.unsqueeze(1).broadcast_to([128, CH, JD])
                ve.tensor_mul(prod, u_ch, vb)
                p4 = prod.rearrange("p g (d j) -> p g d j", d=D)
                t1 = t_pool.tile([128, CH, 8, J], BF16, tag=tag + "t1")
                ve.tensor_add(t1, p4[:, :, 0:8, :], p4[:, :, 8:16, :])
                t2 = t_pool.tile([128, CH, 4, J], BF16, tag=tag + "t2")
                ve.tensor_add(t2, t1[:, :, 0:4, :], t1[:, :, 4:8, :])
                t3 = t_pool.tile([128, CH, 2, J], BF16, tag=tag + "t3")
                ve.tensor_add(t3, t2[:, :, 0:2, :], t2[:, :, 2:4, :])
                tt = t_pool.tile([128, CH, J], BF16, tag=tag + "tt")
                ve.tensor_add(tt.unsqueeze(2), t3[:, :, 0:1, :], t3[:, :, 1:2, :])
                if not emit_exp:
                    return tt
                eT = rp.tile([128, CH, J], BF16, tag=tag + "eT")
                nc.scalar.activation(eT, tt, AF.Exp)
                return eT

            def back_dve(k, eT):
                # softmax over j (no max subtraction; logits are tiny)
                se = rp.tile([128, CH], F32, tag="se")
                nc.vector.reduce_sum(out=se, in_=eT, axis=AX.X)
                r = rp.tile([128, CH], F32, tag="r")
                nc.vector.reciprocal(r, se)
                c = rp.tile([128, CH, J], BF16, tag="c")
                nc.vector.tensor_mul(c, eT, r.unsqueeze(2).broadcast_to([128, CH, J]))
                y = ry.tile([128, CH, D, J], BF16, tag="y")
                cb = c.unsqueeze(2).broadcast_to([128, CH, D, J])
                nc.vector.tensor_mul(y, u_ch_of(k).rearrange("p g (d j) -> p g d j", d=D), cb)
                s_mms(k, y)

            def softmax_dve(eT):
                # softmax normalization on DVE (shared by pool chunks)
                se = rp.tile([128, CH], F32, tag="Pse")
                nc.vector.reduce_sum(out=se, in_=eT, axis=AX.X)
                r = rp.tile([128, CH], F32, tag="Pr")
                nc.vector.reciprocal(r, se)
                c = rp.tile([128, CH, J], BF16, tag="Pc")
                nc.vector.tensor_mul(c, eT, r.unsqueeze(2).broadcast_to([128, CH, J]))
                return c

            def y_pool(k, c):
                y = pp1.tile([128, CH, D, J], BF16, tag="Pprod")
                cb = c.unsqueeze(2).broadcast_to([128, CH, D, J])
                nc.gpsimd.tensor_mul(y, u_ch_of(k).rearrange("p g (d j) -> p g d j", d=D), cb)
                return y

            # pool chunks' heavy front work starts at pass begin, overlapping
            # the DVE chunks; their softmax is slotted into the DVE stream at
            # staggered points matching when each pool front finishes, and
            # their s-matmuls go late in the PE stream so they never block
            # the DVE chunks' s accumulation.
            ptt = {}
            for k in range(ND, NCH):
                ptt[k] = front(k, nc.gpsimd, pp1, "P", emit_exp=False)
            pool_y_slot = {3: ND, 6: ND + 1, 9: ND + 2}
            pool_smm_slot = {10: ND, 12: ND + 1}
            pool_ys = {}
            eTs = {}
            for t in range(ND + 1):
                if t < ND:
                    eTs[t] = front(t, nc.vector, rp1, "")
                if t >= 1:
                    back_dve(t - 1, eTs.pop(t - 1))
                pk = pool_y_slot.get(t)
                if pk is not None:
                    eTp = rp.tile([128, CH, J], BF16, tag="PeT")
                    nc.scalar.activation(eTp, ptt.pop(pk), AF.Exp)
                    pool_ys[pk] = y_pool(pk, softmax_dve(eTp))
                pk = pool_smm_slot.get(t)
                if pk is not None:
                    s_mms(pk, pool_ys.pop(pk))
            s_mms(NCH - 1, pool_ys.pop(NCH - 1))
            nc.vector.tensor_copy(out=s_sb, in_=s_ps)
        if not final:
            v2 = small.tile([8, JD], F32, tag="v2")
            squash_j(s_sb, v2)
            nc.vector.tensor_add(V, V, v2)
            refresh_v_exp()
        else:
            vout = small.tile([8, JD], F32, tag="vout")
            squash_d(s_sb, vout)
            nc.sync.dma_start(out=out, in_=vout)

    routing_pass(final=False)   # iteration 2 (uses V=v1)
    routing_pass(final=True)    # final (uses V=v1+v2)
    ctx.close()


def build_module(n_in=2048, b_loc=8, num_devices=8, enable_asserts=False):
    nc = bacc.Bacc("TRN2", target_bir_lowering=False, debug=False,
                   num_devices=num_devices, enable_asserts=enable_asserts)
    G = n_in // 8
    w2 = nc.dram_tensor("w2", [G, 8, E, JD], BF16, kind="ExternalInput").ap()
    xc = nc.dram_tensor("xc", [128, G, 8], BF16, kind="ExternalInput").ap()
    d1 = nc.dram_tensor("d1", [128, 8], BF16, kind="ExternalInput").ap()
    ds = nc.dram_tensor("ds", [128, 8], BF16, kind="ExternalInput").ap()
    mk = nc.dram_tensor("mk", [128, 8], BF16, kind="ExternalInput").ap()
    out = nc.dram_tensor("out", [b_loc, JD], F32, kind="ExternalOutput").ap()
    with tile.TileContext(nc) as tc:
        emit_capsule(tc, w2, xc, d1, ds, mk, out, n_in=n_in, b_loc=b_loc)
    nc.compile()
    return nc


def host_prep_w(weight, n_in):
    # weight [1, N, J, D, E] -> w2 [G, 8, E, J*D] with free layout (d, j)
    w2 = np.ascontiguousarray(weight[0].transpose(0, 3, 2, 1))  # [N, E, D, J]
    return w2.reshape(n_in // 8, 8, E, JD).astype(ml_dtypes.bfloat16)


def host_prep_xc(xs, n_in):
    # xs [b_loc, N, E] -> xc [128, G, 8]: [(di, e), g, b]
    G = n_in // 8
    t = xs.reshape(8, G, 8, E).transpose(2, 3, 1, 0)  # [di, e, g, b]
    return np.ascontiguousarray(t.reshape(128, G, 8)).astype(ml_dtypes.bfloat16)


def host_prep_deltas():
    p = np.arange(128)
    bofp = (p // 8) % 8
    d1 = np.zeros((128, 8), np.float32)
    ds = np.zeros((128, 8), np.float32)
    d1[p, bofp] = 1.0 / 32.0
    ds[p, bofp] = 1.0
    # mask for block-diag x build: mk[(di,e), di'] = (di == di')
    di = p // 16
    mkm = np.zeros((128, 8), np.float32)
    mkm[p, di] = 1.0
    return (d1.astype(ml_dtypes.bfloat16), ds.astype(ml_dtypes.bfloat16),
            mkm.astype(ml_dtypes.bfloat16))


_CACHE = {}
LAST_EXEC_NS = None


def kernel(x, weight, trace=False):
    B, N_in = 64, 2048
    n_cores = 8
    b_loc = B // n_cores
    key = (N_in, b_loc, n_cores)
    if key not in _CACHE:
        _CACHE[key] = build_module(n_in=N_in, b_loc=b_loc, num_devices=n_cores)
    nc = _CACHE[key]

    x = np.asarray(x, dtype=np.float32)
    weight = np.asarray(weight, dtype=np.float32)
    w2 = host_prep_w(weight, N_in)
    d1, ds, mkm = host_prep_deltas()
    in_maps = []
    for c in range(n_cores):
        xs = np.ascontiguousarray(x[c * b_loc:(c + 1) * b_loc, :, 0, :])
        in_maps.append({
            "w2": w2,
            "xc": host_prep_xc(xs, N_in),
            "d1": d1,
            "ds": ds,
            "mk": mkm,
        })
    global LAST_EXEC_NS
    res = run_bass_kernel_spmd(nc, in_maps, core_ids=list(range(n_cores)),
                               trace=trace)
    LAST_EXEC_NS = res.exec_time_ns
    outs = [r["out"].reshape(b_loc, D, J).transpose(0, 2, 1) for r in res.results]
    return np.ascontiguousarray(np.concatenate(outs, axis=0))


# revision 17
# speedup vs baseline: 1.0083x; 1.0083x over previous
"""CapsuleLayer (dynamic routing, ROUTING_ITER=2) Bass/Tile kernel for TRN2.

Contract: kernel(x, weight) takes FULL inputs
  x:      [64, 2048, 1, 16] f32
  weight: [1, 2048, 32, 16, 16] f32
returns FULL output [64, 32, 16] f32.

Sharding: data-parallel over batch B=64 across 8 cores (8 per core),
weight replicated. Self-contained: hardcodes shapes, imports only
numpy/ml_dtypes/concourse.

v2 layout/engine plan (per core, b_loc=8):
  u layout A: SBUF [p = g2*64 + b*8 + di, f = gh*512 + d*32 + j]
    where i = g*8 + di, g = 2*gh + g2  (G=256 groups, GH=128 pairs).
  phase 1 (DMA-bound ~95us): stream W chunks; build block-diag x
    stationaries on-chip (mask multiply, DVE); u-matmuls + interleaved
    s1 delta-matmuls on PE; PSUM->SBUF u copies mostly on Act.
  routing pass (x2): per chunk of CH=8 gh: prod=u*v (TT), d-tree,
    exp on Act, Z-reduce, c=eT*r, y=c*u; chunks 0..12 on DVE,
    13..15 on Pool(gpsimd); s accumulated with delta matmuls on PE.
"""

from contextlib import ExitStack

import ml_dtypes
import numpy as np

import concourse.bacc as bacc
import concourse.bass as bass
import concourse.mybir as mybir
import concourse.tile as tile
from concourse.bass_utils import run_bass_kernel_spmd

F32 = mybir.dt.float32
BF16 = mybir.dt.bfloat16
AF = mybir.ActivationFunctionType
AX = mybir.AxisListType

EPS = 1e-8
J, D, E = 32, 16, 16
JD = J * D  # 512


def emit_capsule(tc, w2, xc, d1, ds, mk, out, n_in, b_loc=8):
    """Emit the per-core capsule program.

    DRAM tensors (APs):
      w2  [G, 8, E, JD] bf16  weight, host-permuted to [g, di, e, (d j)]
      xc  [128, G, 8]   bf16  x compact: [(di,e), g, b]
      d1  [128, 8] bf16       delta matrix * (1/32)  (s1 accumulation)
      ds  [128, 8] bf16       delta matrix * 1.0     (s2/s3 accumulation)
      mk  [128, 8] bf16       block-diag mask: mk[(di,e), di'] = (di==di')
      out [b_loc, JD] f32     squash(s3) output, (d,j) layout
    """
    nc = tc.nc
    assert b_loc == 8
    G = n_in // 8          # 256
    GH = G // 2            # 128
    GDMA = 16              # groups per W DMA chunk
    NWCH = G // GDMA       # 16 W chunks
    CH = 8                 # gh per routing chunk
    NCH = GH // CH         # 16 routing chunks
    N_POOL = 3             # routing chunks on gpsimd (the last ones)

    ctx = ExitStack()
    singles = ctx.enter_context(tc.tile_pool(name="singles", bufs=1))
    small = ctx.enter_context(tc.tile_pool(name="small", bufs=1))
    dramp = ctx.enter_context(tc.tile_pool(name="dram_scratch", bufs=1, space="DRAM"))

    u_sb = singles.tile([128, GH * JD], BF16)
    d1_sb = singles.tile([128, 8], BF16)
    ds_sb = singles.tile([128, 8], BF16)
    mk_sb = singles.tile([128, 8], BF16)
    nc.sync.dma_start(out=d1_sb, in_=d1)
    nc.sync.dma_start(out=ds_sb, in_=ds)
    nc.sync.dma_start(out=mk_sb, in_=mk)
    v_exp = singles.tile([128, JD], BF16)
    V = singles.tile([8, JD], F32)      # running sum of v vectors
    s_sb = singles.tile([8, JD], F32)
    vscr = dramp.tile([8, JD], BF16)    # DRAM bounce buffer for v broadcast
    eps8 = singles.tile([8, 1], F32)
    nc.vector.memset(eps8, EPS)

    # ---------- squash helpers (all on 8 partitions, tiny) ----------
    def squash_j(s_in, v_out):
        # v = squash(s, axis=j):  sq[b,d] = sum_j s^2;  v = s*sq/((1+sq)*sqrt(sq+eps))
        t2 = small.tile([8, JD], F32, tag="sqt2")
        nc.vector.tensor_mul(t2, s_in, s_in)
        sv = small.tile([8, 4, J], F32, tag="sqv")
        sq, a, t3, w = sv[:, 0, :D], sv[:, 1, :D], sv[:, 2, :D], sv[:, 3, :D]
        nc.vector.reduce_sum(out=sq, in_=t2.rearrange("p (d j) -> p d j", d=D), axis=AX.X)
        nc.scalar.activation(a, sq, AF.Sqrt, bias=eps8)
        nc.vector.tensor_mul(t3, sq, a)
        nc.vector.tensor_add(t3, t3, a)          # a*(1+sq)
        nc.vector.reciprocal(w, t3)
        nc.vector.tensor_mul(w, w, sq)           # sq/((1+sq)a)
        wb = w.unsqueeze(2).broadcast_to([8, D, J])
        nc.vector.tensor_mul(v_out.rearrange("p (d j) -> p d j", d=D),
                             s_in.rearrange("p (d j) -> p d j", d=D), wb)

    def squash_d(s_in, v_out):
        # v = squash(s, axis=d): sq[b,j] = sum_d s^2
        t2 = small.tile([8, JD], F32, tag="sqt2")
        nc.vector.tensor_mul(t2, s_in, s_in)
        sv = small.tile([8, 4, J], F32, tag="sqv")
        sq, a, t3, w = sv[:, 0, :], sv[:, 1, :], sv[:, 2, :], sv[:, 3, :]
        nc.vector.reduce_sum(out=sq, in_=t2.rearrange("p (d j) -> p j d", d=D), axis=AX.X)
        nc.scalar.activation(a, sq, AF.Sqrt, bias=eps8)
        nc.vector.tensor_mul(t3, sq, a)
        nc.vector.tensor_add(t3, t3, a)
        nc.vector.reciprocal(w, t3)
        nc.vector.tensor_mul(w, w, sq)
        wb = w.unsqueeze(1).broadcast_to([8, D, J])
        nc.vector.tensor_mul(v_out.rearrange("p (d j) -> p d j", d=D),
                             s_in.rearrange("p (d j) -> p d j", d=D), wb)

    def refresh_v_exp():
        vb = small.tile([8, JD], BF16, tag="vb")
        nc.vector.tensor_copy(out=vb, in_=V)
        nc.sync.dma_start(out=vscr, in_=vb)
        src = vscr.unsqueeze(1).broadcast_to([8, 8, JD])
        for g2 in range(2):
            nc.sync.dma_start(out=v_exp[g2 * 64:(g2 + 1) * 64, :], in_=src)

    # ---------- phase 1: W pass (u = W @ x), s1 accumulation ----------
    with tc.tile_pool(name="wp", bufs=3) as wp, \
         tc.tile_pool(name="xbp", bufs=2) as xbp, \
         tc.tile_pool(name="up", bufs=3, space="PSUM") as up, \
         tc.tile_pool(name="sp", bufs=1, space="PSUM") as sp:
        s1_ps = sp.tile([8, JD], F32)
        x_sb = wp.tile([128, G, 8], BF16, tag="xsb")
        nc.sync.dma_start(out=x_sb, in_=xc)
        mkb = mk_sb.unsqueeze(1).unsqueeze(1).broadcast_to([128, GDMA, 8, 8])
        for ci in range(NWCH):
            wt = wp.tile([128, GDMA, JD], BF16, tag="wt")
            # src: dims (k=(di,e) merged, g, jd)
            wsrc = w2[ci * GDMA:(ci + 1) * GDMA].rearrange("g di e f -> (di e) g f")
            nc.sync.dma_start(out=wt, in_=wsrc)
            # on-chip block-diag x stationary for this chunk:
            # xt[(di,e), g_loc, b, di'] = x[(di,e), g, b] * (di == di')
            xt = xbp.tile([128, GDMA, 8, 8], BF16, tag="xt")
            xb = x_sb[:, ci * GDMA:(ci + 1) * GDMA, :].unsqueeze(3) \
                .broadcast_to([128, GDMA, 8, 8])
            nc.gpsimd.tensor_mul(xt, xb, mkb)
            xt2 = xt.rearrange("p g b q -> p g (b q)")
            for gq in range(GDMA // 4):
                pt = up.tile([128, 2 * JD], F32, tag="upt")
                for idx in range(4):
                    gl = gq * 4 + idx
                    nc.tensor.matmul(
                        pt[(gl % 2) * 64:(gl % 2) * 64 + 64,
                           (idx // 2) * JD:(idx // 2) * JD + JD],
                        xt2[:, gl, :], wt[:, gl, :], start=True, stop=True)
                gh0 = (ci * GDMA) // 2 + gq * 2
                if gq % 4 == 0:
                    nc.vector.tensor_copy(out=u_sb[:, gh0 * JD:(gh0 + 2) * JD], in_=pt)
                else:
                    nc.scalar.copy(out=u_sb[:, gh0 * JD:(gh0 + 2) * JD], in_=pt)
                for gh in (gh0, gh0 + 1):
                    nc.tensor.matmul(s1_ps, d1_sb, u_sb[:, gh * JD:(gh + 1) * JD],
                                     start=(gh == 0), stop=(gh == GH - 1))
        nc.vector.tensor_copy(out=s_sb, in_=s1_ps)

    squash_j(s_sb, V)      # V = v1
    refresh_v_exp()

    # ---------- routing pass (T = u.V, softmax, s = sum_i c*u) ----------
    ND = NCH - N_POOL  # DVE routing chunks; pool gets the rest

    def routing_pass(final):
        # rp1: engine-local scratch (consumed by the next op on the same
        # in-order engine) -> single buffer per tag suffices.
        # ry: y tiles, consumed cross-engine by PE -> double buffered.
        # rp: small cross-engine tiles (eT from Act, softmax pieces).
        with tc.tile_pool(name="rp1", bufs=1) as rp1, \
             tc.tile_pool(name="ry", bufs=2) as ry, \
             tc.tile_pool(name="rp", bufs=3) as rp, \
             tc.tile_pool(name="pp1", bufs=1) as pp1, \
             tc.tile_pool(name="ppy", bufs=1) as ppy, \
             tc.tile_pool(name="spp", bufs=1, space="PSUM") as spp:
            s_ps = spp.tile([8, JD], F32)
            smm = [0]  # emission-order counter for s-accumulation matmuls

            def u_ch_of(k):
                fs = k * CH * JD
                return u_sb[:, fs:fs + CH * JD].rearrange("p (g f) -> p g f", g=CH)

            def s_mms(k, y):
                for q in range(CH):
                    nc.tensor.matmul(s_ps, ds_sb,
                                     y[:, q].rearrange("p d j -> p (d j)"),
                                     start=(smm[0] == 0), stop=(smm[0] == GH - 1))
                    smm[0] += 1

            def front(k, ve, t_pool, tag, emit_exp=True):
                # prod = u * v, then tree-reduce over d (16->8->4->2->1), bf16
                u_ch = u_ch_of(k)
                prod = t_pool.tile([128, CH, JD], BF16, tag=tag + "prod")
                vb = v_exp.unsqueeze(1).broadcast_to([128, CH, JD])
                ve.tensor_mul(prod, u_ch, vb)
                p4 = prod.rearrange("p g (d j) -> p g d j", d=D)
                t1 = t_pool.tile([128, CH, 8, J], BF16, tag=tag + "t1")
                ve.tensor_add(t1, p4[:, :, 0:8, :], p4[:, :, 8:16, :])
                t2 = t_pool.tile([128, CH, 4, J], BF16, tag=tag + "t2")
                ve.tensor_add(t2, t1[:, :, 0:4, :], t1[:, :, 4:8, :])
                t3 = t_pool.tile([128, CH, 2, J], BF16, tag=tag + "t3")
                ve.tensor_add(t3, t2[:, :, 0:2, :], t2[:, :, 2:4, :])
                tt = rp.tile([128, CH, J], BF16, tag=tag + "tt")
                ve.tensor_add(tt.unsqueeze(2), t3[:, :, 0:1, :], t3[:, :, 1:2, :])
                if not emit_exp:
                    return tt
                eT = rp.tile([128, CH, J], BF16, tag=tag + "eT")
                nc.scalar.activation(eT, tt, AF.Exp)
                return eT

            def back_dve(k, eT):
                # softmax over j (no max subtraction; logits are tiny)
                se = rp.tile([128, CH], F32, tag="se")
                nc.vector.reduce_sum(out=se, in_=eT, axis=AX.X)
                r = rp.tile([128, CH], F32, tag="r")
                nc.vector.reciprocal(r, se)
                c = rp.tile([128, CH, J], BF16, tag="c")
                nc.vector.tensor_mul(c, eT, r.unsqueeze(2).broadcast_to([128, CH, J]))
                y = ry.tile([128, CH, D, J], BF16, tag="y")
                cb = c.unsqueeze(2).broadcast_to([128, CH, D, J])
                nc.vector.tensor_mul(y, u_ch_of(k).rearrange("p g (d j) -> p g d j", d=D), cb)
                s_mms(k, y)

            def softmax_dve(eT):
                # softmax normalization on DVE (shared by pool chunks)
                se = rp.tile([128, CH], F32, tag="Pse")
                nc.vector.reduce_sum(out=se, in_=eT, axis=AX.X)
                r = rp.tile([128, CH], F32, tag="Pr")
                nc.vector.reciprocal(r, se)
                c = rp.tile([128, CH, J], BF16, tag="Pc")
                nc.vector.tensor_mul(c, eT, r.unsqueeze(2).broadcast_to([128, CH, J]))
                return c

            def y_pool(k, c):
                y = ppy.tile([128, CH, D, J], BF16, tag="Py")
                cb = c.unsqueeze(2).broadcast_to([128, CH, D, J])
                nc.gpsimd.tensor_mul(y, u_ch_of(k).rearrange("p g (d j) -> p g d j", d=D), cb)
                return y

            # pool chunks' heavy front work starts at pass begin, overlapping
            # the DVE chunks; their softmax is slotted into the DVE stream at
            # staggered points matching when each pool front finishes, and
            # their s-matmuls go late in the PE stream so they never block
            # the DVE chunks' s accumulation.
            ptt = {}
            for k in range(ND, NCH):
                ptt[k] = front(k, nc.gpsimd, pp1, "P", emit_exp=False)
            pool_y_slot = {3: ND, 5: ND + 1, 7: ND + 2}
            pool_smm_slot = {9: ND, 11: ND + 1}
            pool_ys = {}
            eTs = {}
            for t in range(ND + 1):
                if t < ND:
                    eTs[t] = front(t, nc.vector, rp1, "")
                if t >= 1:
                    back_dve(t - 1, eTs.pop(t - 1))
                pk = pool_y_slot.get(t)
                if pk is not None:
                    eTp = rp.tile([128, CH, J], BF16, tag="PeT")
                    nc.scalar.activation(eTp, ptt.pop(pk), AF.Exp)
                    pool_ys[pk] = y_pool(pk, softmax_dve(eTp))
                pk = pool_smm_slot.get(t)
                if pk is not None:
                    s_mms(pk, pool_ys.pop(pk))
            s_mms(NCH - 1, pool_ys.pop(NCH - 1))
            nc.vector.tensor_copy(out=s_sb, in_=s_ps)
        if not final:
            v2 = small.tile([8, JD], F32, tag="v2")
            squash_j(s_sb, v2)
            nc.vector.tensor_add(V, V, v2)
            refresh_v_exp()
        else:
            vout = small.tile([8, JD], F32, tag="vout")
            squash_d(s_sb, vout)
            nc.sync.dma_start(out=out, in_=vout)

    routing_pass(final=False)   # iteration 2 (uses V=v1)
    routing_pass(final=True)    # final (uses V=v1+v2)
    ctx.close()


def build_module(n_in=2048, b_loc=8, num_devices=8, enable_asserts=False):
    nc = bacc.Bacc("TRN2", target_bir_lowering=False, debug=False,
                   num_devices=num_devices, enable_asserts=enable_asserts)
    G = n_in // 8
    w2 = nc.dram_tensor("w2", [G, 8, E, JD], BF16, kind="ExternalInput").ap()
    xc = nc.dram_tensor("xc", [128, G, 8], BF16, kind="ExternalInput").ap()
    d1 = nc.dram_tensor("d1", [128, 8], BF16, kind="ExternalInput").ap()
    ds = nc.dram_tensor("ds", [128, 8], BF16, kind="ExternalInput").ap()
    mk = nc.dram_tensor("mk", [128, 8], BF16, kind="ExternalInput").ap()
    out = nc.dram_tensor("out", [b_loc, JD], F32, kind="ExternalOutput").ap()
    with tile.TileContext(nc) as tc:
        emit_capsule(tc, w2, xc, d1, ds, mk, out, n_in=n_in, b_loc=b_loc)
    nc.compile()
    return nc


def host_prep_w(weight, n_in):
    # weight [1, N, J, D, E] -> w2 [G, 8, E, J*D] with free layout (d, j)
    w2 = np.ascontiguousarray(weight[0].transpose(0, 3, 2, 1))  # [N, E, D, J]
    return w2.reshape(n_in // 8, 8, E, JD).astype(ml_dtypes.bfloat16)


def host_prep_xc(xs, n_in):
    # xs [b_loc, N, E] -> xc [128, G, 8]: [(di, e), g, b]
    G = n_in // 8
    t = xs.reshape(8, G, 8, E).transpose(2, 3, 1, 0)  # [di, e, g, b]
    return np.ascontiguousarray(t.reshape(128, G, 8)).astype(ml_dtypes.bfloat16)


def host_prep_deltas():
    p = np.arange(128)
    bofp = (p // 8) % 8
    d1 = np.zeros((128, 8), np.float32)
    ds = np.zeros((128, 8), np.float32)
    d1[p, bofp] = 1.0 / 32.0
    ds[p, bofp] = 1.0
    # mask for block-diag x build: mk[(di,e), di'] = (di == di')
    di = p // 16
    mkm = np.zeros((128, 8), np.float32)
    mkm[p, di] = 1.0
    return (d1.astype(ml_dtypes.bfloat16), ds.astype(ml_dtypes.bfloat16),
            mkm.astype(ml_dtypes.bfloat16))


_CACHE = {}
LAST_EXEC_NS = None


def kernel(x, weight, trace=False):
    B, N_in = 64, 2048
    n_cores = 8
    b_loc = B // n_cores
    key = (N_in, b_loc, n_cores)
    if key not in _CACHE:
        _CACHE[key] = build_module(n_in=N_in, b_loc=b_loc, num_devices=n_cores)
    nc = _CACHE[key]

    x = np.asarray(x, dtype=np.float32)
    weight = np.asarray(weight, dtype=np.float32)
    w2 = host_prep_w(weight, N_in)
    d1, ds, mkm = host_prep_deltas()
    in_maps = []
    for c in range(n_cores):
        xs = np.ascontiguousarray(x[c * b_loc:(c + 1) * b_loc, :, 0, :])
        in_maps.append({
            "w2": w2,
            "xc": host_prep_xc(xs, N_in),
            "d1": d1,
            "ds": ds,
            "mk": mkm,
        })
    global LAST_EXEC_NS
    res = run_bass_kernel_spmd(nc, in_maps, core_ids=list(range(n_cores)),
                               trace=trace)
    LAST_EXEC_NS = res.exec_time_ns
    outs = [r["out"].reshape(b_loc, D, J).transpose(0, 2, 1) for r in res.results]
    return np.ascontiguousarray(np.concatenate(outs, axis=0))


# revision 21
# speedup vs baseline: 1.1023x; 1.0932x over previous
"""CapsuleLayer (dynamic routing, ROUTING_ITER=2) Bass/Tile kernel for TRN2.

Contract: kernel(x, weight) takes FULL inputs
  x:      [64, 2048, 1, 16] f32
  weight: [1, 2048, 32, 16, 16] f32
returns FULL output [64, 32, 16] f32.

Sharding: data-parallel over batch B=64 across 8 cores (8 per core),
weight replicated. Self-contained: hardcodes shapes, imports only
numpy/ml_dtypes/concourse.

v2 layout/engine plan (per core, b_loc=8):
  u layout A: SBUF [p = g2*64 + b*8 + di, f = gh*512 + d*32 + j]
    where i = g*8 + di, g = 2*gh + g2  (G=256 groups, GH=128 pairs).
  phase 1 (DMA-bound ~95us): stream W chunks; build block-diag x
    stationaries on-chip (mask multiply, DVE); u-matmuls + interleaved
    s1 delta-matmuls on PE; PSUM->SBUF u copies mostly on Act.
  routing pass (x2): per chunk of CH=8 gh: prod=u*v (TT), d-tree,
    exp on Act, Z-reduce, c=eT*r, y=c*u; chunks 0..12 on DVE,
    13..15 on Pool(gpsimd); s accumulated with delta matmuls on PE.
"""

from contextlib import ExitStack

import ml_dtypes
import numpy as np

import concourse.bacc as bacc
import concourse.bass as bass
import concourse.mybir as mybir
import concourse.tile as tile
from concourse.bass_utils import run_bass_kernel_spmd

F32 = mybir.dt.float32
BF16 = mybir.dt.bfloat16
AF = mybir.ActivationFunctionType
AX = mybir.AxisListType

EPS = 1e-8
J, D, E = 32, 16, 16
JD = J * D  # 512


def emit_capsule(tc, w2, xc, d1, ds, dsT, mk, out, n_in, b_loc=8):
    """Emit the per-core capsule program.

    DRAM tensors (APs):
      w2  [G, 8, E, JD] bf16  weight, host-permuted to [g, di, e, (d j)]
      xc  [128, G, 8]   bf16  x compact: [(di,e), g, b]
      d1  [128, 8] bf16       delta matrix * (1/32)  (s1 accumulation)
      ds  [128, 8] bf16       delta matrix * 1.0     (s2/s3 accumulation)
      dsT [8, 128] bf16       transposed delta (v broadcast stationary)
      mk  [128, 8] bf16       block-diag mask: mk[(di,e), di'] = (di==di')
      out [b_loc, JD] f32     squash(s3) output, (d,j) layout
    """
    nc = tc.nc
    assert b_loc == 8
    G = n_in // 8          # 256
    GH = G // 2            # 128
    GDMA = 16              # groups per W DMA chunk
    NWCH = G // GDMA       # 16 W chunks
    CH = 8                 # gh per routing chunk
    NCH = GH // CH         # 16 routing chunks
    N_POOL = 3             # routing chunks on gpsimd (the last ones)

    ctx = ExitStack()
    singles = ctx.enter_context(tc.tile_pool(name="singles", bufs=1))
    small = ctx.enter_context(tc.tile_pool(name="small", bufs=1))

    u_sb = singles.tile([128, GH * JD], BF16)

    d1_sb = singles.tile([128, 8], BF16)
    ds_sb = singles.tile([128, 8], BF16)
    mk_sb = singles.tile([128, 8], BF16)
    nc.sync.dma_start(out=d1_sb, in_=d1)
    nc.sync.dma_start(out=ds_sb, in_=ds)
    nc.sync.dma_start(out=mk_sb, in_=mk)
    v_exp = singles.tile([128, JD], BF16)
    V = singles.tile([8, JD], F32)      # running sum of v vectors
    s_sb = singles.tile([8, JD], F32)
    dsT_sb = singles.tile([8, 128], BF16)
    nc.sync.dma_start(out=dsT_sb, in_=dsT)
    eps8 = singles.tile([8, 1], F32)
    nc.vector.memset(eps8, EPS)
    vrepp = ctx.enter_context(tc.tile_pool(name="vrep", bufs=1, space="PSUM"))
    vrep_ps = vrepp.tile([128, JD], F32)

    # ---------- squash helpers (all on 8 partitions, tiny) ----------
    def squash_j(s_in, v_out):
        # v = squash(s, axis=j):  sq[b,d] = sum_j s^2;  v = s*sq/((1+sq)*sqrt(sq+eps))
        t2 = small.tile([8, JD], F32, tag="sqt2")
        nc.vector.tensor_mul(t2, s_in, s_in)
        sv = small.tile([8, 4, J], F32, tag="sqv")
        sq, a, t3, w = sv[:, 0, :D], sv[:, 1, :D], sv[:, 2, :D], sv[:, 3, :D]
        nc.vector.reduce_sum(out=sq, in_=t2.rearrange("p (d j) -> p d j", d=D), axis=AX.X)
        nc.scalar.activation(a, sq, AF.Sqrt, bias=eps8)
        nc.vector.tensor_mul(t3, sq, a)
        nc.vector.tensor_add(t3, t3, a)          # a*(1+sq)
        nc.vector.reciprocal(w, t3)
        nc.vector.tensor_mul(w, w, sq)           # sq/((1+sq)a)
        wb = w.unsqueeze(2).broadcast_to([8, D, J])
        nc.vector.tensor_mul(v_out.rearrange("p (d j) -> p d j", d=D),
                             s_in.rearrange("p (d j) -> p d j", d=D), wb)

    def squash_d(s_in, v_out):
        # v = squash(s, axis=d): sq[b,j] = sum_d s^2
        t2 = small.tile([8, JD], F32, tag="sqt2")
        nc.vector.tensor_mul(t2, s_in, s_in)
        sv = small.tile([8, 4, J], F32, tag="sqv")
        sq, a, t3, w = sv[:, 0, :], sv[:, 1, :], sv[:, 2, :], sv[:, 3, :]
        nc.vector.reduce_sum(out=sq, in_=t2.rearrange("p (d j) -> p j d", d=D), axis=AX.X)
        nc.scalar.activation(a, sq, AF.Sqrt, bias=eps8)
        nc.vector.tensor_mul(t3, sq, a)
        nc.vector.tensor_add(t3, t3, a)
        nc.vector.reciprocal(w, t3)
        nc.vector.tensor_mul(w, w, sq)
        wb = w.unsqueeze(1).broadcast_to([8, D, J])
        nc.vector.tensor_mul(v_out.rearrange("p (d j) -> p d j", d=D),
                             s_in.rearrange("p (d j) -> p d j", d=D), wb)

    def refresh_v_exp():
        # replicate V across all 128 partitions with a ones-stationary matmul
        vb = small.tile([8, JD], BF16, tag="vb")
        nc.vector.tensor_copy(out=vb, in_=V)
        nc.tensor.matmul(vrep_ps, dsT_sb, vb, start=True, stop=True)
        nc.vector.tensor_copy(out=v_exp, in_=vrep_ps)

    # ---------- phase 1: W pass (u = W @ x), s1 accumulation ----------
    with tc.tile_pool(name="wp", bufs=3) as wp, \
         tc.tile_pool(name="xbp", bufs=2) as xbp, \
         tc.tile_pool(name="up", bufs=3, space="PSUM") as up, \
         tc.tile_pool(name="sp", bufs=1, space="PSUM") as sp:
        s1_ps = sp.tile([8, JD], F32)
        x_sb = wp.tile([128, G, 8], BF16, tag="xsb")
        nc.sync.dma_start(out=x_sb, in_=xc)
        mkb = mk_sb.unsqueeze(1).unsqueeze(1).broadcast_to([128, GDMA, 8, 8])
        for ci in range(NWCH):
            wt = wp.tile([128, GDMA, JD], BF16, tag="wt")
            # src: dims (k=(di,e) merged, g, jd)
            wsrc = w2[ci * GDMA:(ci + 1) * GDMA].rearrange("g di e f -> (di e) g f")
            nc.sync.dma_start(out=wt, in_=wsrc)
            # on-chip block-diag x stationary for this chunk:
            # xt[(di,e), g_loc, b, di'] = x[(di,e), g, b] * (di == di')
            xt = xbp.tile([128, GDMA, 8, 8], BF16, tag="xt")
            xb = x_sb[:, ci * GDMA:(ci + 1) * GDMA, :].unsqueeze(3) \
                .broadcast_to([128, GDMA, 8, 8])
            nc.gpsimd.tensor_mul(xt, xb, mkb)
            xt2 = xt.rearrange("p g b q -> p g (b q)")
            for gq in range(GDMA // 4):
                pt = up.tile([128, 2 * JD], F32, tag="upt")
                for idx in range(4):
                    gl = gq * 4 + idx
                    nc.tensor.matmul(
                        pt[(gl % 2) * 64:(gl % 2) * 64 + 64,
                           (idx // 2) * JD:(idx // 2) * JD + JD],
                        xt2[:, gl, :], wt[:, gl, :], start=True, stop=True)
                gh0 = (ci * GDMA) // 2 + gq * 2
                if gq % 4 == 0:
                    nc.vector.tensor_copy(out=u_sb[:, gh0 * JD:(gh0 + 2) * JD], in_=pt)
                else:
                    nc.scalar.copy(out=u_sb[:, gh0 * JD:(gh0 + 2) * JD], in_=pt)
                for gh in (gh0, gh0 + 1):
                    nc.tensor.matmul(s1_ps, d1_sb, u_sb[:, gh * JD:(gh + 1) * JD],
                                     start=(gh == 0), stop=(gh == GH - 1))
        nc.vector.tensor_copy(out=s_sb, in_=s1_ps)

    squash_j(s_sb, V)      # V = v1
    refresh_v_exp()

    # ---------- routing pass (T = u.V, softmax, s = sum_i c*u) ----------
    ND = NCH - N_POOL        # DVE routing chunks; pool gets the rest
    CHP = 4                  # gh per pool unit (smaller for slot alignment)
    NPU = N_POOL * CH // CHP  # pool units
    POOL_GH0 = ND * CH       # first gh handled by the pool engine

    def routing_pass(final):
        # rp1: engine-local scratch (consumed by the next op on the same
        # in-order engine) -> single buffer per tag suffices.
        # ry: y tiles, consumed cross-engine by PE -> double buffered.
        # rp: small cross-engine tiles (eT from Act, softmax pieces).
        with tc.tile_pool(name="rp1", bufs=1) as rp1, \
             tc.tile_pool(name="ry", bufs=2) as ry, \
             tc.tile_pool(name="rp", bufs=3) as rp, \
             tc.tile_pool(name="pp1", bufs=1) as pp1, \
             tc.tile_pool(name="ppy", bufs=1) as ppy, \
             tc.tile_pool(name="spp", bufs=1, space="PSUM") as spp:
            s_ps = spp.tile([8, JD], F32)
            smm = [0]  # emission-order counter for s-accumulation matmuls

            def u_ch_of(gh0, n):
                fs = gh0 * JD
                return u_sb[:, fs:fs + n * JD].rearrange("p (g f) -> p g f", g=n)

            def s_mms(gh0, n, y):
                for q in range(n):
                    nc.tensor.matmul(s_ps, ds_sb,
                                     y[:, q].rearrange("p d j -> p (d j)"),
                                     start=(smm[0] == 0), stop=(smm[0] == GH - 1))
                    smm[0] += 1

            def front(gh0, n, ve, t_pool, tag, emit_exp=True):
                # prod = u * v, then tree-reduce over d (16->8->4->2->1), bf16
                u_ch = u_ch_of(gh0, n)
                prod = t_pool.tile([128, n, JD], BF16, tag=tag + "prod")
                vb = v_exp.unsqueeze(1).broadcast_to([128, n, JD])
                ve.tensor_mul(prod, u_ch, vb)
                p4 = prod.rearrange("p g (d j) -> p g d j", d=D)
                t1 = t_pool.tile([128, n, 8, J], BF16, tag=tag + "t1")
                ve.tensor_add(t1, p4[:, :, 0:8, :], p4[:, :, 8:16, :])
                t2 = t_pool.tile([128, n, 4, J], BF16, tag=tag + "t2")
                ve.tensor_add(t2, t1[:, :, 0:4, :], t1[:, :, 4:8, :])
                t3 = t_pool.tile([128, n, 2, J], BF16, tag=tag + "t3")
                ve.tensor_add(t3, t2[:, :, 0:2, :], t2[:, :, 2:4, :])
                tt = rp.tile([128, n, J], BF16, tag=tag + "tt")
                ve.tensor_add(tt.unsqueeze(2), t3[:, :, 0:1, :], t3[:, :, 1:2, :])
                if not emit_exp:
                    return tt
                eT = rp.tile([128, n, J], BF16, tag=tag + "eT")
                nc.scalar.activation(eT, tt, AF.Exp)
                return eT

            def softmax(n, eT, tag):
                # softmax over j (no max subtraction; logits are tiny)
                se = rp.tile([128, n], F32, tag=tag + "se")
                nc.vector.reduce_sum(out=se, in_=eT, axis=AX.X)
                r = rp.tile([128, n], F32, tag=tag + "r")
                nc.vector.reciprocal(r, se)
                c = rp.tile([128, n, J], BF16, tag=tag + "c")
                nc.vector.tensor_mul(c, eT, r.unsqueeze(2).broadcast_to([128, n, J]))
                return c

            def back_dve(k, eT):
                c = softmax(CH, eT, "")
                y = ry.tile([128, CH, D, J], BF16, tag="y")
                cb = c.unsqueeze(2).broadcast_to([128, CH, D, J])
                nc.vector.tensor_mul(
                    y, u_ch_of(k * CH, CH).rearrange("p g (d j) -> p g d j", d=D), cb)
                s_mms(k * CH, CH, y)

            def y_pool(gh0, c):
                y = ppy.tile([128, CHP, D, J], BF16, tag="Py")
                cb = c.unsqueeze(2).broadcast_to([128, CHP, D, J])
                nc.gpsimd.tensor_mul(
                    y, u_ch_of(gh0, CHP).rearrange("p g (d j) -> p g d j", d=D), cb)
                return y

            # pool work is emitted as NPU small units so its fronts finish at
            # a cadence matching the DVE chunk pipeline; each unit's softmax is
            # slotted into the DVE stream right when its front is done, and
            # pool s-matmuls go late in the PE stream so they never block the
            # DVE chunks' s accumulation.
            ptt = {}
            for u in range(NPU):
                ptt[u] = front(POOL_GH0 + u * CHP, CHP, nc.gpsimd, pp1, "P",
                               emit_exp=False)
            pool_ys = {}
            eTs = {}
            for t in range(ND + 1):
                if t < ND:
                    eTs[t] = front(t * CH, CH, nc.vector, rp1, "")
                if t >= 1:
                    back_dve(t - 1, eTs.pop(t - 1))
                u = t - 2
                if 0 <= u < NPU:  # pool softmax + y at slots t=2..2+NPU-1
                    eTp = rp.tile([128, CHP, J], BF16, tag="PeT")
                    nc.scalar.activation(eTp, ptt.pop(u), AF.Exp)
                    pool_ys[u] = y_pool(POOL_GH0 + u * CHP, softmax(CHP, eTp, "P"))
                u = t - 8
                if 0 <= u < NPU - 1:  # pool s-matmuls at slots t=8..8+NPU-2
                    s_mms(POOL_GH0 + u * CHP, CHP, pool_ys.pop(u))
            s_mms(POOL_GH0 + (NPU - 1) * CHP, CHP, pool_ys.pop(NPU - 1))
            nc.vector.tensor_copy(out=s_sb, in_=s_ps)
        if not final:
            v2 = small.tile([8, JD], F32, tag="v2")
            squash_j(s_sb, v2)
            nc.vector.tensor_add(V, V, v2)
            refresh_v_exp()
        else:
            vout = small.tile([8, JD], F32, tag="vout")
            squash_d(s_sb, vout)
            nc.sync.dma_start(out=out, in_=vout)

    routing_pass(final=False)   # iteration 2 (uses V=v1)
    routing_pass(final=True)    # final (uses V=v1+v2)
    ctx.close()


def build_module(n_in=2048, b_loc=8, num_devices=8, enable_asserts=False):
    nc = bacc.Bacc("TRN2", target_bir_lowering=False, debug=False,
                   num_devices=num_devices, enable_asserts=enable_asserts)
    G = n_in // 8
    w2 = nc.dram_tensor("w2", [G, 8, E, JD], BF16, kind="ExternalInput").ap()
    xc = nc.dram_tensor("xc", [128, G, 8], BF16, kind="ExternalInput").ap()
    d1 = nc.dram_tensor("d1", [128, 8], BF16, kind="ExternalInput").ap()
    ds = nc.dram_tensor("ds", [128, 8], BF16, kind="ExternalInput").ap()
    dsT = nc.dram_tensor("dsT", [8, 128], BF16, kind="ExternalInput").ap()
    mk = nc.dram_tensor("mk", [128, 8], BF16, kind="ExternalInput").ap()
    out = nc.dram_tensor("out", [b_loc, JD], F32, kind="ExternalOutput").ap()
    with tile.TileContext(nc) as tc:
        emit_capsule(tc, w2, xc, d1, ds, dsT, mk, out, n_in=n_in, b_loc=b_loc)
    nc.compile()
    return nc


def host_prep_w(weight, n_in):
    # weight [1, N, J, D, E] -> w2 [G, 8, E, J*D] with free layout (d, j)
    w2 = np.ascontiguousarray(weight[0].transpose(0, 3, 2, 1))  # [N, E, D, J]
    return w2.reshape(n_in // 8, 8, E, JD).astype(ml_dtypes.bfloat16)


def host_prep_xc(xs, n_in):
    # xs [b_loc, N, E] -> xc [128, G, 8]: [(di, e), g, b]
    G = n_in // 8
    t = xs.reshape(8, G, 8, E).transpose(2, 3, 1, 0)  # [di, e, g, b]
    return np.ascontiguousarray(t.reshape(128, G, 8)).astype(ml_dtypes.bfloat16)


def host_prep_deltas():
    p = np.arange(128)
    bofp = (p // 8) % 8
    d1 = np.zeros((128, 8), np.float32)
    ds = np.zeros((128, 8), np.float32)
    d1[p, bofp] = 1.0 / 32.0
    ds[p, bofp] = 1.0
    # mask for block-diag x build: mk[(di,e), di'] = (di == di')
    di = p // 16
    mkm = np.zeros((128, 8), np.float32)
    mkm[p, di] = 1.0
    return (d1.astype(ml_dtypes.bfloat16), ds.astype(ml_dtypes.bfloat16),
            np.ascontiguousarray(ds.T).astype(ml_dtypes.bfloat16),
            mkm.astype(ml_dtypes.bfloat16))


_CACHE = {}
LAST_EXEC_NS = None


def kernel(x, weight, trace=False):
    B, N_in = 64, 2048
    n_cores = 8
    b_loc = B // n_cores
    key = (N_in, b_loc, n_cores)
    if key not in _CACHE:
        _CACHE[key] = build_module(n_in=N_in, b_loc=b_loc, num_devices=n_cores)
    nc = _CACHE[key]

    x = np.asarray(x, dtype=np.float32)
    weight = np.asarray(weight, dtype=np.float32)
    w2 = host_prep_w(weight, N_in)
    d1, ds, dsT, mkm = host_prep_deltas()
    in_maps = []
    for c in range(n_cores):
        xs = np.ascontiguousarray(x[c * b_loc:(c + 1) * b_loc, :, 0, :])
        in_maps.append({
            "w2": w2,
            "xc": host_prep_xc(xs, N_in),
            "d1": d1,
            "ds": ds,
            "dsT": dsT,
            "mk": mkm,
        })
    global LAST_EXEC_NS
    res = run_bass_kernel_spmd(nc, in_maps, core_ids=list(range(n_cores)),
                               trace=trace)
    LAST_EXEC_NS = res.exec_time_ns
    outs = [r["out"].reshape(b_loc, D, J).transpose(0, 2, 1) for r in res.results]
    return np.ascontiguousarray(np.concatenate(outs, axis=0))


# revision 22
# speedup vs baseline: 1.1114x; 1.0083x over previous
"""CapsuleLayer (dynamic routing, ROUTING_ITER=2) Bass/Tile kernel for TRN2.

Contract: kernel(x, weight) takes FULL inputs
  x:      [64, 2048, 1, 16] f32
  weight: [1, 2048, 32, 16, 16] f32
returns FULL output [64, 32, 16] f32.

Sharding: data-parallel over batch B=64 across 8 cores (8 per core),
weight replicated. Self-contained: hardcodes shapes, imports only
numpy/ml_dtypes/concourse.

v2 layout/engine plan (per core, b_loc=8):
  u layout A: SBUF [p = g2*64 + b*8 + di, f = gh*512 + d*32 + j]
    where i = g*8 + di, g = 2*gh + g2  (G=256 groups, GH=128 pairs).
  phase 1 (DMA-bound ~95us): stream W chunks; build block-diag x
    stationaries on-chip (mask multiply, DVE); u-matmuls + interleaved
    s1 delta-matmuls on PE; PSUM->SBUF u copies mostly on Act.
  routing pass (x2): per chunk of CH=8 gh: prod=u*v (TT), d-tree,
    exp on Act, Z-reduce, c=eT*r, y=c*u; chunks 0..12 on DVE,
    13..15 on Pool(gpsimd); s accumulated with delta matmuls on PE.
"""

from contextlib import ExitStack

import ml_dtypes
import numpy as np

import concourse.bacc as bacc
import concourse.bass as bass
import concourse.mybir as mybir
import concourse.tile as tile
from concourse.bass_utils import run_bass_kernel_spmd

F32 = mybir.dt.float32
BF16 = mybir.dt.bfloat16
AF = mybir.ActivationFunctionType
AX = mybir.AxisListType

EPS = 1e-8
J, D, E = 32, 16, 16
JD = J * D  # 512


def emit_capsule(tc, w2, xc, d1, ds, dsT, mk, out, n_in, b_loc=8):
    """Emit the per-core capsule program.

    DRAM tensors (APs):
      w2  [G, 8, E, JD] bf16  weight, host-permuted to [g, di, e, (d j)]
      xc  [128, G, 8]   bf16  x compact: [(di,e), g, b]
      d1  [128, 8] bf16       delta matrix * (1/32)  (s1 accumulation)
      ds  [128, 8] bf16       delta matrix * 1.0     (s2/s3 accumulation)
      dsT [8, 128] bf16       transposed delta (v broadcast stationary)
      mk  [128, 8] bf16       block-diag mask: mk[(di,e), di'] = (di==di')
      out [b_loc, JD] f32     squash(s3) output, (d,j) layout
    """
    nc = tc.nc
    assert b_loc == 8
    G = n_in // 8          # 256
    GH = G // 2            # 128
    GDMA = 16              # groups per W DMA chunk
    NWCH = G // GDMA       # 16 W chunks
    CH = 8                 # gh per routing chunk
    NCH = GH // CH         # 16 routing chunks
    N_POOL = 3             # routing chunks on gpsimd (the last ones)

    ctx = ExitStack()
    singles = ctx.enter_context(tc.tile_pool(name="singles", bufs=1))
    small = ctx.enter_context(tc.tile_pool(name="small", bufs=1))

    u_sb = singles.tile([128, GH * JD], BF16)

    d1_sb = singles.tile([128, 8], BF16)
    ds_sb = singles.tile([128, 8], BF16)
    mk_sb = singles.tile([128, 8], BF16)
    v_exp = singles.tile([128, JD], BF16)
    V = singles.tile([8, JD], F32)      # running sum of v vectors
    s_sb = singles.tile([8, JD], F32)
    dsT_sb = singles.tile([8, 128], BF16)
    eps8 = singles.tile([8, 1], F32)
    nc.vector.memset(eps8, EPS)
    vrepp = ctx.enter_context(tc.tile_pool(name="vrep", bufs=1, space="PSUM"))
    vrep_ps = vrepp.tile([128, JD], F32)

    # ---------- squash helpers (all on 8 partitions, tiny) ----------
    def squash_j(s_in, v_out):
        # v = squash(s, axis=j):  sq[b,d] = sum_j s^2;  v = s*sq/((1+sq)*sqrt(sq+eps))
        t2 = small.tile([8, JD], F32, tag="sqt2")
        nc.vector.tensor_mul(t2, s_in, s_in)
        sv = small.tile([8, 4, J], F32, tag="sqv")
        sq, a, t3, w = sv[:, 0, :D], sv[:, 1, :D], sv[:, 2, :D], sv[:, 3, :D]
        nc.vector.reduce_sum(out=sq, in_=t2.rearrange("p (d j) -> p d j", d=D), axis=AX.X)
        nc.scalar.activation(a, sq, AF.Sqrt, bias=eps8)
        nc.vector.tensor_mul(t3, sq, a)
        nc.vector.tensor_add(t3, t3, a)          # a*(1+sq)
        nc.vector.reciprocal(w, t3)
        nc.vector.tensor_mul(w, w, sq)           # sq/((1+sq)a)
        wb = w.unsqueeze(2).broadcast_to([8, D, J])
        nc.vector.tensor_mul(v_out.rearrange("p (d j) -> p d j", d=D),
                             s_in.rearrange("p (d j) -> p d j", d=D), wb)

    def squash_d(s_in, v_out):
        # v = squash(s, axis=d): sq[b,j] = sum_d s^2
        t2 = small.tile([8, JD], F32, tag="sqt2")
        nc.vector.tensor_mul(t2, s_in, s_in)
        sv = small.tile([8, 4, J], F32, tag="sqv")
        sq, a, t3, w = sv[:, 0, :], sv[:, 1, :], sv[:, 2, :], sv[:, 3, :]
        nc.vector.reduce_sum(out=sq, in_=t2.rearrange("p (d j) -> p j d", d=D), axis=AX.X)
        nc.scalar.activation(a, sq, AF.Sqrt, bias=eps8)
        nc.vector.tensor_mul(t3, sq, a)
        nc.vector.tensor_add(t3, t3, a)
        nc.vector.reciprocal(w, t3)
        nc.vector.tensor_mul(w, w, sq)
        wb = w.unsqueeze(1).broadcast_to([8, D, J])
        nc.vector.tensor_mul(v_out.rearrange("p (d j) -> p d j", d=D),
                             s_in.rearrange("p (d j) -> p d j", d=D), wb)

    def refresh_v_exp():
        # replicate V across all 128 partitions with a ones-stationary matmul
        vb = small.tile([8, JD], BF16, tag="vb")
        nc.vector.tensor_copy(out=vb, in_=V)
        nc.tensor.matmul(vrep_ps, dsT_sb, vb, start=True, stop=True)
        nc.vector.tensor_copy(out=v_exp, in_=vrep_ps)

    # ---------- phase 1: W pass (u = W @ x), s1 accumulation ----------
    with tc.tile_pool(name="wp", bufs=3) as wp, \
         tc.tile_pool(name="xbp", bufs=2) as xbp, \
         tc.tile_pool(name="up", bufs=3, space="PSUM") as up, \
         tc.tile_pool(name="sp", bufs=1, space="PSUM") as sp:
        s1_ps = sp.tile([8, JD], F32)
        x_sb = wp.tile([128, G, 8], BF16, tag="xsb")
        mkb = mk_sb.unsqueeze(1).unsqueeze(1).broadcast_to([128, GDMA, 8, 8])
        for ci in range(NWCH):
            wt = wp.tile([128, GDMA, JD], BF16, tag="wt")
            # src: dims (k=(di,e) merged, g, jd)
            wsrc = w2[ci * GDMA:(ci + 1) * GDMA].rearrange("g di e f -> (di e) g f")
            nc.sync.dma_start(out=wt, in_=wsrc)
            if ci == 0:
                # small constants ride behind the first W chunk on the DMA queue
                nc.sync.dma_start(out=x_sb, in_=xc)
                nc.sync.dma_start(out=d1_sb, in_=d1)
                nc.sync.dma_start(out=ds_sb, in_=ds)
                nc.sync.dma_start(out=mk_sb, in_=mk)
                nc.sync.dma_start(out=dsT_sb, in_=dsT)
            # on-chip block-diag x stationary for this chunk:
            # xt[(di,e), g_loc, b, di'] = x[(di,e), g, b] * (di == di')
            xt = xbp.tile([128, GDMA, 8, 8], BF16, tag="xt")
            xb = x_sb[:, ci * GDMA:(ci + 1) * GDMA, :].unsqueeze(3) \
                .broadcast_to([128, GDMA, 8, 8])
            nc.gpsimd.tensor_mul(xt, xb, mkb)
            xt2 = xt.rearrange("p g b q -> p g (b q)")
            for gq in range(GDMA // 4):
                pt = up.tile([128, 2 * JD], F32, tag="upt")
                for idx in range(4):
                    gl = gq * 4 + idx
                    nc.tensor.matmul(
                        pt[(gl % 2) * 64:(gl % 2) * 64 + 64,
                           (idx // 2) * JD:(idx // 2) * JD + JD],
                        xt2[:, gl, :], wt[:, gl, :], start=True, stop=True)
                gh0 = (ci * GDMA) // 2 + gq * 2
                if gq % 4 == 0:
                    nc.vector.tensor_copy(out=u_sb[:, gh0 * JD:(gh0 + 2) * JD], in_=pt)
                else:
                    nc.scalar.copy(out=u_sb[:, gh0 * JD:(gh0 + 2) * JD], in_=pt)
                for gh in (gh0, gh0 + 1):
                    nc.tensor.matmul(s1_ps, d1_sb, u_sb[:, gh * JD:(gh + 1) * JD],
                                     start=(gh == 0), stop=(gh == GH - 1))
        nc.vector.tensor_copy(out=s_sb, in_=s1_ps)

    squash_j(s_sb, V)      # V = v1
    refresh_v_exp()

    # ---------- routing pass (T = u.V, softmax, s = sum_i c*u) ----------
    ND = NCH - N_POOL        # DVE routing chunks; pool gets the rest
    CHP = 4                  # gh per pool unit (smaller for slot alignment)
    NPU = N_POOL * CH // CHP  # pool units
    POOL_GH0 = ND * CH       # first gh handled by the pool engine

    def routing_pass(final):
        # rp1: engine-local scratch (consumed by the next op on the same
        # in-order engine) -> single buffer per tag suffices.
        # ry: y tiles, consumed cross-engine by PE -> double buffered.
        # rp: small cross-engine tiles (eT from Act, softmax pieces).
        with tc.tile_pool(name="rp1", bufs=1) as rp1, \
             tc.tile_pool(name="ry", bufs=2) as ry, \
             tc.tile_pool(name="rp", bufs=3) as rp, \
             tc.tile_pool(name="pp1", bufs=1) as pp1, \
             tc.tile_pool(name="ppy", bufs=1) as ppy, \
             tc.tile_pool(name="spp", bufs=1, space="PSUM") as spp:
            s_ps = spp.tile([8, JD], F32)
            smm = [0]  # emission-order counter for s-accumulation matmuls

            def u_ch_of(gh0, n):
                fs = gh0 * JD
                return u_sb[:, fs:fs + n * JD].rearrange("p (g f) -> p g f", g=n)

            def s_mms(gh0, n, y):
                for q in range(n):
                    nc.tensor.matmul(s_ps, ds_sb,
                                     y[:, q].rearrange("p d j -> p (d j)"),
                                     start=(smm[0] == 0), stop=(smm[0] == GH - 1))
                    smm[0] += 1

            def front(gh0, n, ve, t_pool, tag, emit_exp=True):
                # prod = u * v, then tree-reduce over d (16->8->4->2->1), bf16
                u_ch = u_ch_of(gh0, n)
                prod = t_pool.tile([128, n, JD], BF16, tag=tag + "prod")
                vb = v_exp.unsqueeze(1).broadcast_to([128, n, JD])
                ve.tensor_mul(prod, u_ch, vb)
                p4 = prod.rearrange("p g (d j) -> p g d j", d=D)
                t1 = t_pool.tile([128, n, 8, J], BF16, tag=tag + "t1")
                ve.tensor_add(t1, p4[:, :, 0:8, :], p4[:, :, 8:16, :])
                t2 = t_pool.tile([128, n, 4, J], BF16, tag=tag + "t2")
                ve.tensor_add(t2, t1[:, :, 0:4, :], t1[:, :, 4:8, :])
                t3 = t_pool.tile([128, n, 2, J], BF16, tag=tag + "t3")
                ve.tensor_add(t3, t2[:, :, 0:2, :], t2[:, :, 2:4, :])
                tt = rp.tile([128, n, J], BF16, tag=tag + "tt")
                ve.tensor_add(tt.unsqueeze(2), t3[:, :, 0:1, :], t3[:, :, 1:2, :])
                if not emit_exp:
                    return tt
                eT = rp.tile([128, n, J], BF16, tag=tag + "eT")
                nc.scalar.activation(eT, tt, AF.Exp)
                return eT

            def softmax(n, eT, tag):
                # softmax over j (no max subtraction; logits are tiny)
                se = rp.tile([128, n], F32, tag=tag + "se")
                nc.vector.reduce_sum(out=se, in_=eT, axis=AX.X)
                r = rp.tile([128, n], F32, tag=tag + "r")
                nc.vector.reciprocal(r, se)
                c = rp.tile([128, n, J], BF16, tag=tag + "c")
                nc.vector.tensor_mul(c, eT, r.unsqueeze(2).broadcast_to([128, n, J]))
                return c

            def back_dve(k, eT):
                c = softmax(CH, eT, "")
                y = ry.tile([128, CH, D, J], BF16, tag="y")
                cb = c.unsqueeze(2).broadcast_to([128, CH, D, J])
                nc.vector.tensor_mul(
                    y, u_ch_of(k * CH, CH).rearrange("p g (d j) -> p g d j", d=D), cb)
                s_mms(k * CH, CH, y)

            def y_pool(gh0, c):
                y = ppy.tile([128, CHP, D, J], BF16, tag="Py")
                cb = c.unsqueeze(2).broadcast_to([128, CHP, D, J])
                nc.gpsimd.tensor_mul(
                    y, u_ch_of(gh0, CHP).rearrange("p g (d j) -> p g d j", d=D), cb)
                return y

            # pool work is emitted as NPU small units so its fronts finish at
            # a cadence matching the DVE chunk pipeline; each unit's softmax is
            # slotted into the DVE stream right when its front is done, and
            # pool s-matmuls go late in the PE stream so they never block the
            # DVE chunks' s accumulation.
            ptt = {}
            for u in range(NPU):
                ptt[u] = front(POOL_GH0 + u * CHP, CHP, nc.gpsimd, pp1, "P",
                               emit_exp=False)
            pool_ys = {}
            eTs = {}
            for t in range(ND + 1):
                if t < ND:
                    eTs[t] = front(t * CH, CH, nc.vector, rp1, "")
                if t >= 1:
                    back_dve(t - 1, eTs.pop(t - 1))
                u = t - 3
                if 0 <= u < NPU:  # pool softmax + y at slots t=3..3+NPU-1
                    eTp = rp.tile([128, CHP, J], BF16, tag="PeT")
                    nc.scalar.activation(eTp, ptt.pop(u), AF.Exp)
                    pool_ys[u] = y_pool(POOL_GH0 + u * CHP, softmax(CHP, eTp, "P"))
                u = t - 8
                if 0 <= u < NPU - 1:  # pool s-matmuls at slots t=8..8+NPU-2
                    s_mms(POOL_GH0 + u * CHP, CHP, pool_ys.pop(u))
            s_mms(POOL_GH0 + (NPU - 1) * CHP, CHP, pool_ys.pop(NPU - 1))
            nc.vector.tensor_copy(out=s_sb, in_=s_ps)
        if not final:
            v2 = small.tile([8, JD], F32, tag="v2")
            squash_j(s_sb, v2)
            nc.vector.tensor_add(V, V, v2)
            refresh_v_exp()
        else:
            vout = small.tile([8, JD], F32, tag="vout")
            squash_d(s_sb, vout)
            nc.sync.dma_start(out=out, in_=vout)

    routing_pass(final=False)   # iteration 2 (uses V=v1)
    routing_pass(final=True)    # final (uses V=v1+v2)
    ctx.close()


def build_module(n_in=2048, b_loc=8, num_devices=8, enable_asserts=False):
    nc = bacc.Bacc("TRN2", target_bir_lowering=False, debug=False,
                   num_devices=num_devices, enable_asserts=enable_asserts)
    G = n_in // 8
    w2 = nc.dram_tensor("w2", [G, 8, E, JD], BF16, kind="ExternalInput").ap()
    xc = nc.dram_tensor("xc", [128, G, 8], BF16, kind="ExternalInput").ap()
    d1 = nc.dram_tensor("d1", [128, 8], BF16, kind="ExternalInput").ap()
    ds = nc.dram_tensor("ds", [128, 8], BF16, kind="ExternalInput").ap()
    dsT = nc.dram_tensor("dsT", [8, 128], BF16, kind="ExternalInput").ap()
    mk = nc.dram_tensor("mk", [128, 8], BF16, kind="ExternalInput").ap()
    out = nc.dram_tensor("out", [b_loc, JD], F32, kind="ExternalOutput").ap()
    with tile.TileContext(nc) as tc:
        emit_capsule(tc, w2, xc, d1, ds, dsT, mk, out, n_in=n_in, b_loc=b_loc)
    nc.compile()
    return nc


def host_prep_w(weight, n_in):
    # weight [1, N, J, D, E] -> w2 [G, 8, E, J*D] with free layout (d, j)
    w2 = np.ascontiguousarray(weight[0].transpose(0, 3, 2, 1))  # [N, E, D, J]
    return w2.reshape(n_in // 8, 8, E, JD).astype(ml_dtypes.bfloat16)


def host_prep_xc(xs, n_in):
    # xs [b_loc, N, E] -> xc [128, G, 8]: [(di, e), g, b]
    G = n_in // 8
    t = xs.reshape(8, G, 8, E).transpose(2, 3, 1, 0)  # [di, e, g, b]
    return np.ascontiguousarray(t.reshape(128, G, 8)).astype(ml_dtypes.bfloat16)


def host_prep_deltas():
    p = np.arange(128)
    bofp = (p // 8) % 8
    d1 = np.zeros((128, 8), np.float32)
    ds = np.zeros((128, 8), np.float32)
    d1[p, bofp] = 1.0 / 32.0
    ds[p, bofp] = 1.0
    # mask for block-diag x build: mk[(di,e), di'] = (di == di')
    di = p // 16
    mkm = np.zeros((128, 8), np.float32)
    mkm[p, di] = 1.0
    return (d1.astype(ml_dtypes.bfloat16), ds.astype(ml_dtypes.bfloat16),
            np.ascontiguousarray(ds.T).astype(ml_dtypes.bfloat16),
            mkm.astype(ml_dtypes.bfloat16))


_CACHE = {}
LAST_EXEC_NS = None


def kernel(x, weight, trace=False):
    B, N_in = 64, 2048
    n_cores = 8
    b_loc = B // n_cores
    key = (N_in, b_loc, n_cores)
    if key not in _CACHE:
        _CACHE[key] = build_module(n_in=N_in, b_loc=b_loc, num_devices=n_cores)
    nc = _CACHE[key]

    x = np.asarray(x, dtype=np.float32)
    weight = np.asarray(weight, dtype=np.float32)
    w2 = host_prep_w(weight, N_in)
    d1, ds, dsT, mkm = host_prep_deltas()
    in_maps = []
    for c in range(n_cores):
        xs = np.ascontiguousarray(x[c * b_loc:(c + 1) * b_loc, :, 0, :])
        in_maps.append({
            "w2": w2,
            "xc": host_prep_xc(xs, N_in),
            "d1": d1,
            "ds": ds,
            "dsT": dsT,
            "mk": mkm,
        })
    global LAST_EXEC_NS
    res = run_bass_kernel_spmd(nc, in_maps, core_ids=list(range(n_cores)),
                               trace=trace)
    LAST_EXEC_NS = res.exec_time_ns
    outs = [r["out"].reshape(b_loc, D, J).transpose(0, 2, 1) for r in res.results]
    return np.ascontiguousarray(np.concatenate(outs, axis=0))
